# revision 5
# baseline (speedup 1.0000x reference)
"""Trainium2 Bass kernel for nn_DelayExpansionLayer (histogram_binning).

Computation: per-channel mean of layer_output [64,256,56,56] over (B,H,W),
round to 1e-6, nearest-key lookup in a sorted 1024-entry table, max over
channels, scale by (in_ch*out_ch)/512, broadcast to (56,56).

Strategy (data-parallel over batch, 8 NeuronCores):
  - Each core gets 8 batches = [8,256,56,56] (25.7 MB) and computes
    per-channel partial sums [256] on-device (DMA-bound reduction).
  - Host combines the 8 partial-sum vectors (the tiny [C] all-reduce),
    then does the O(C+K) lookup/max/broadcast epilogue.

Per-core device kernel:
  input  x [8, 128, 2, 3136] f32  (batch, partition, chan-pair, spatial)
         -> each batch slice is one fully-contiguous 3.2 MB DMA
  8x DVE reduce_sum along free dims -> stats[128, 2, 8]
  final reduce over batches -> out [128, 2]  (channel c = 2*p + j)
"""

import sys
import types

import numpy as np

N_CORES = 8
B_FULL, C, H, W = 64, 256, 56, 56
HW = H * W
B_LOCAL = B_FULL // N_CORES
N_CHUNKS = 17  # 8 (j=0) + 7 (j=1) + 2 (j=1, b=7 split in half)
SCALE_DENOM = 32 * 16

# Set by a test harness to enable NTFF tracing of the SPMD run.
TRACE = False
TRACE_TMPDIR = None
LAST_RESULTS = None

_CACHE = {}


def _ensure_axon_hooks_shim():
    """bass_utils' axon trace path imports antenv.axon_hooks; provide a
    no-op shim when the environment's antenv package lacks it."""
    try:
        import antenv.axon_hooks  # noqa: F401
        return
    except ImportError:
        pass
    import antenv

    mod = types.ModuleType("antenv.axon_hooks")
    _hook = [None]
    mod.set_axon_ntff_profile_hook = lambda h: _hook.__setitem__(0, h)
    mod.get_axon_ntff_profile_hook = lambda: _hook[0]
    sys.modules["antenv.axon_hooks"] = mod
    antenv.axon_hooks = mod


def _build():
    if "nc" in _CACHE:
        return _CACHE["nc"]
    import concourse.bacc as bacc
    import concourse.tile as tile
    from concourse import mybir

    nc = bacc.Bacc(
        "TRN2",
        target_bir_lowering=False,
        debug=False,
        enable_asserts=False,
        num_devices=N_CORES,
    )
    x = nc.dram_tensor(
        "x", [B_LOCAL, 128, 2, HW], mybir.dt.float32, kind="ExternalInput"
    ).ap()
    # 17 partial sums per partition: j=0 chunks (b=0..7) -> cols 0..7,
    # j=1 chunks (b=0..6) -> cols 8..14, b=7 j=1 halves -> cols 15,16.
    # channel c = 2p + j; host sums the matching columns.
    out = nc.dram_tensor(
        "out", [128, N_CHUNKS], mybir.dt.float32, kind="ExternalOutput"
    ).ap()

    # chunk list: (src_ap_fn, col). Order = DMA issue order (arrival order).
    with tile.TileContext(nc) as tc:
        with (
            tc.tile_pool(name="inp", bufs=6) as inp,
            tc.tile_pool(name="stats", bufs=1) as sp,
        ):
            stats = sp.tile([128, N_CHUNKS], mybir.dt.float32)

            def do_chunk(src, col, width):
                t = inp.tile([128, HW], mybir.dt.float32, tag="t")
                nc.sync.dma_start(t[:, 0:width], src)
                nc.vector.reduce_sum(
                    stats[:, col : col + 1],
                    t[:, 0:width],
                    axis=mybir.AxisListType.X,
                )

            for b in range(B_LOCAL):
                do_chunk(x[b, :, 0, :], b, HW)
                if b < B_LOCAL - 1:
                    do_chunk(x[b, :, 1, :], 8 + b, HW)
                else:
                    half = HW // 2
                    do_chunk(x[b, :, 1, 0:half], 15, half)
                    do_chunk(x[b, :, 1, half:HW], 16, HW - half)
            nc.sync.dma_start(out[:], stats[:])

    nc.compile()
    _CACHE["nc"] = nc
    return nc


def kernel(layer_output, delay_keys, delay_values, in_channels, out_channels):
    global LAST_RESULTS
    _ensure_axon_hooks_shim()
    from concourse.bass_utils import run_bass_kernel_spmd

    x = np.ascontiguousarray(np.asarray(layer_output, dtype=np.float32))
    assert x.shape == (B_FULL, C, H, W), x.shape
    # shard over batch; view channels as (partition, pair): c = 2*p + j
    xr = x.reshape(N_CORES, B_LOCAL, 128, 2, HW)
    in_maps = [{"x": xr[k]} for k in range(N_CORES)]

    nc = _build()
    kwargs = {}
    if TRACE:
        kwargs.update(trace=True, tmpdir=TRACE_TMPDIR)
    res = run_bass_kernel_spmd(nc, in_maps, core_ids=list(range(N_CORES)), **kwargs)
    LAST_RESULTS = res

    # tiny [C] all-reduce of the per-core partial sums
    parts = np.stack(
        [res.results[k]["out"] for k in range(N_CORES)]
    )  # [8, 128, 17]
    s_even = parts[:, :, 0:8].sum(axis=(0, 2), dtype=np.float32)  # ch 2p
    s_odd = parts[:, :, 8:17].sum(axis=(0, 2), dtype=np.float32)  # ch 2p+1
    sums = np.stack([s_even, s_odd], axis=1).reshape(C)  # channel c = 2p+j
    means = sums / np.float32(B_FULL * HW)
    means = np.round(means * np.float32(1e6)) / np.float32(1e6)

    keys = np.asarray(delay_keys, dtype=np.float32)
    values = np.asarray(delay_values, dtype=np.float32)
    K = keys.shape[0]
    idx = np.searchsorted(keys, means)
    lo = np.clip(idx - 1, 0, K - 1)
    hi = np.clip(idx, 0, K - 1)
    pick_hi = np.abs(keys[hi] - means) < np.abs(keys[lo] - means)
    nearest = np.where(pick_hi, hi, lo)
    merged = np.float32(values[nearest].max())

    scale = np.float32(
        (int(np.asarray(in_channels)) * int(np.asarray(out_channels))) / SCALE_DENOM
    )
    return np.full((H, W), merged, dtype=np.float32) * scale


# revision 8
# speedup vs baseline: 1.1694x; 1.1694x over previous
"""Trainium2 Bass kernel for nn_DelayExpansionLayer (histogram_binning).

Computation: per-channel mean of layer_output [64,256,56,56] over (B,H,W),
round to 1e-6, nearest-key lookup in a sorted 1024-entry table, max over
channels, scale by (in_ch*out_ch)/512, broadcast to (56,56).

Strategy (data-parallel over batch, 8 NeuronCores):
  - Each core gets 8 batches = [8,256,56,56] (25.7 MB) and computes
    per-channel partial sums [256] on-device (DMA-bound reduction).
  - Host combines the 8 partial-sum vectors (the tiny [C] all-reduce),
    then does the O(C+K) lookup/max/broadcast epilogue.

Per-core device kernel:
  input  x [8, 128, 2, 3136] f32  (batch, partition, chan-pair, spatial)
         -> each batch slice is one fully-contiguous 3.2 MB DMA
  8x DVE reduce_sum along free dims -> stats[128, 2, 8]
  final reduce over batches -> out [128, 2]  (channel c = 2*p + j)
"""

import sys
import types

import numpy as np

N_CORES = 8
B_FULL, C, H, W = 64, 256, 56, 56
HW = H * W
B_LOCAL = B_FULL // N_CORES
N_CHUNKS = 18  # batches 0-6: 2 half-spatial chunks each; batch 7: 4 quarters
SCALE_DENOM = 32 * 16

# Set by a test harness to enable NTFF tracing of the SPMD run.
TRACE = False
TRACE_TMPDIR = None
LAST_RESULTS = None

_CACHE = {}


def _ensure_axon_hooks_shim():
    """bass_utils' axon trace path imports antenv.axon_hooks; provide a
    no-op shim when the environment's antenv package lacks it."""
    try:
        import antenv.axon_hooks  # noqa: F401
        return
    except ImportError:
        pass
    import antenv

    mod = types.ModuleType("antenv.axon_hooks")
    _hook = [None]
    mod.set_axon_ntff_profile_hook = lambda h: _hook.__setitem__(0, h)
    mod.get_axon_ntff_profile_hook = lambda: _hook[0]
    sys.modules["antenv.axon_hooks"] = mod
    antenv.axon_hooks = mod


def _build():
    if "nc" in _CACHE:
        return _CACHE["nc"]
    import concourse.bacc as bacc
    import concourse.tile as tile
    from concourse import mybir

    nc = bacc.Bacc(
        "TRN2",
        target_bir_lowering=False,
        debug=False,
        enable_asserts=False,
        num_devices=N_CORES,
    )
    x = nc.dram_tensor(
        "x", [B_LOCAL, 128, 2, HW], mybir.dt.float32, kind="ExternalInput"
    ).ap()
    # stats[p, j, k]: chunk k's partial sum for channel 2p+j. The reduce
    # output is [128, 2] per chunk (even free dim -> DVE 2x perf mode).
    # Host sums over k (and over cores).
    out = nc.dram_tensor(
        "out", [128, 2, N_CHUNKS], mybir.dt.float32, kind="ExternalOutput"
    ).ap()

    with tile.TileContext(nc) as tc:
        with (
            tc.tile_pool(name="inp", bufs=8) as inp,
            tc.tile_pool(name="stats", bufs=1) as sp,
        ):
            stats = sp.tile([128, 2, N_CHUNKS], mybir.dt.float32)
            col = [0]

            def do_chunk(b, s0, s1):
                w = s1 - s0
                t = inp.tile([128, 2, HW // 2], mybir.dt.float32, tag="t")
                nc.sync.dma_start(t[:, :, 0:w], x[b, :, :, s0:s1])
                nc.vector.reduce_sum(
                    stats[:, :, col[0] : col[0] + 1],
                    t[:, :, 0:w],
                    axis=mybir.AxisListType.X,
                )
                col[0] += 1

            half = HW // 2
            quarter = HW // 4
            for b in range(B_LOCAL - 1):
                do_chunk(b, 0, half)
                do_chunk(b, half, HW)
            for q in range(4):
                do_chunk(B_LOCAL - 1, q * quarter, (q + 1) * quarter)
            assert col[0] == N_CHUNKS
            nc.sync.dma_start(out[:], stats[:])

    nc.compile()
    _CACHE["nc"] = nc
    return nc


def kernel(layer_output, delay_keys, delay_values, in_channels, out_channels):
    global LAST_RESULTS
    _ensure_axon_hooks_shim()
    from concourse.bass_utils import run_bass_kernel_spmd

    x = np.ascontiguousarray(np.asarray(layer_output, dtype=np.float32))
    assert x.shape == (B_FULL, C, H, W), x.shape
    # shard over batch; view channels as (partition, pair): c = 2*p + j
    xr = x.reshape(N_CORES, B_LOCAL, 128, 2, HW)
    in_maps = [{"x": xr[k]} for k in range(N_CORES)]

    nc = _build()
    kwargs = {}
    if TRACE:
        kwargs.update(trace=True, tmpdir=TRACE_TMPDIR)
    res = run_bass_kernel_spmd(nc, in_maps, core_ids=list(range(N_CORES)), **kwargs)
    LAST_RESULTS = res

    # tiny [C] all-reduce of the per-core partial sums
    parts = np.stack(
        [res.results[k]["out"] for k in range(N_CORES)]
    )  # [8, 128, 2, 18]
    sums = parts.sum(axis=(0, 3), dtype=np.float32).reshape(C)  # c = 2p+j
    means = sums / np.float32(B_FULL * HW)
    means = np.round(means * np.float32(1e6)) / np.float32(1e6)

    keys = np.asarray(delay_keys, dtype=np.float32)
    values = np.asarray(delay_values, dtype=np.float32)
    K = keys.shape[0]
    idx = np.searchsorted(keys, means)
    lo = np.clip(idx - 1, 0, K - 1)
    hi = np.clip(idx, 0, K - 1)
    pick_hi = np.abs(keys[hi] - means) < np.abs(keys[lo] - means)
    nearest = np.where(pick_hi, hi, lo)
    merged = np.float32(values[nearest].max())

    scale = np.float32(
        (int(np.asarray(in_channels)) * int(np.asarray(out_channels))) / SCALE_DENOM
    )
    return np.full((H, W), merged, dtype=np.float32) * scale
